# revision 16
# baseline (speedup 1.0000x reference)
"""Trainium2 Bass kernel for nn_ClusteringModel (vq_codebook).

Reference math (R=2, Q=1, c=1, beta=3, Tc=1, Twta=0.1, phi=1.5):
  a = attn/S;  wdist_bc = sum_d a_d (x_bd - w_cd)^2;  r = sqrt(wdist)
  p_comp = softmax_c(-3r | recruited); competed = p_comp * exp(-r) * m
  p_wta  = softmax_c(competed/0.1 | recruited)
  y = 1.5 * (p_wta * competed) @ w_assoc

Kernel algebra (u = raw attn, S = sum u):
  wdist*S = sum_d u x^2 - 2 sum_d u x w + sum_d u w^2  as ONE PSUM
  accumulation: the cross term is a K=256 float32r matmul, u*w^2 rides a
  ones-block lhsT, and the recruitment mask enters additively (+BIG) via a
  K=1 ones-row matmul (which doubles as a partition broadcast).
  r = exp(0.5*ln(wdist)) keeps every ACT func (ln/exp/square) in one
  activation-table set -> a single table load.  E1 = exp(-3r) -> s1;
  v = exp(-4r);  E2 = exp((10v - BIGmask)/s1) -> s2;
  y = 1.5/(s1*s2) * (E2*v) @ w_assoc.

This version is RAW bacc (no TileContext): 4 hand-scheduled engine streams
(Sync/Scalar handle DMA issue, PE, DVE, ACT) with 7 monotonic semaphores
and at most one wait per instruction (extra deps use standalone waits), so
the TRN2 event-semaphore splitting pass has nothing to do and the kernel
carries no scheduler / sem-cleanup overhead.

Sharding: data-parallel over batch (8 cores x 128 rows); w_dist/attn/
w_assoc/mask replicated. Host does layout prep only (transpose/concat).
"""

import sys

if "/opt/trn_rl_repo" not in sys.path:
    sys.path.insert(0, "/opt/trn_rl_repo")

import numpy as np

B, C, D = 1024, 512, 256
N_CORES = 8
BS = B // N_CORES            # 128 batch rows per core
KC = D // 128                # 2 contraction chunks
W = BS + C + 1 + 128         # big row: [xT | wT | u | ones_block]
SM = 3 * C + D + 128         # smalls:  [mask | waT | u | ones_row]
BIG = 1.0e8
EPS_RAW = 0.01

_CACHE = {}
_PATCHED = False


def _apply_env_patches():
    """Collapse the ln/exp activation-table choice onto the combined set so
    the kernel needs exactly one ACT table load."""
    global _PATCHED
    if _PATCHED:
        return
    import copy

    import concourse.bacc as bacc
    import concourse.mybir as mybir

    AF = mybir.ActivationFunctionType
    orig_tables = bacc.get_activation_tables

    def tables_single_ln_exp(module_arch):
        t = copy.deepcopy(orig_tables(module_arch))
        for name, funcs in t.items():
            if name == "natural_log_exp_and_others":
                continue
            funcs.discard(AF.Ln)
            funcs.discard(AF.Exp)
        return t

    bacc.get_activation_tables = tables_single_ln_exp
    _PATCHED = True


def _build(matmul_dt_name="float32r", end_clear=False):
    import dataclasses
    from contextlib import ExitStack

    import concourse.bacc as bacc
    import concourse.mybir as mybir

    _apply_env_patches()

    mdt = getattr(mybir.dt, matmul_dt_name)
    f32 = mybir.dt.float32
    AF = mybir.ActivationFunctionType
    OP = mybir.AluOpType

    def dtv(ap, dt):
        if ap.tensor.dtype == dt:
            return ap
        return dataclasses.replace(ap, tensor=dataclasses.replace(ap.tensor, dtype=dt))

    nc = bacc.Bacc("TRN2", target_bir_lowering=False)

    big = nc.dram_tensor("big", [D, W], mdt, kind="ExternalInput")
    xn = nc.dram_tensor("xn", [BS, D], f32, kind="ExternalInput")
    smalls = nc.dram_tensor("smalls", [1, SM], mdt, kind="ExternalInput")
    y = nc.dram_tensor("y", [BS, 2], f32, kind="ExternalOutput")

    with ExitStack() as ctx:
        e = ctx.enter_context

        # ---- semaphores (monotonic; runtime provides them zeroed) ----
        s_big = e(nc.semaphore("s_big"))
        s_sm = e(nc.semaphore("s_sm"))
        s_xn = e(nc.semaphore("s_xn"))
        s_dve = e(nc.semaphore("s_dve"))
        s_act = e(nc.semaphore("s_act"))
        s_pe = e(nc.semaphore("s_pe"))
        s_out = e(nc.semaphore("s_out"))

        # ---- SBUF ----
        big_sb = e(nc.sbuf_tensor("big_sb", [128, KC, W], mdt))
        sm_sb = e(nc.sbuf_tensor("sm_sb", [1, SM], mdt))
        xn_sb = e(nc.sbuf_tensor("xn_sb", [BS, D], f32))
        warm = e(nc.sbuf_tensor("warm", [1, 1], f32))
        mrow = e(nc.sbuf_tensor("mrow", [1, C], mdt))
        xx = e(nc.sbuf_tensor("xx", [BS, D], f32))
        t1c = e(nc.sbuf_tensor("t1c", [BS, 1], f32))
        scr_t1 = e(nc.sbuf_tensor("scr_t1", [BS, D], f32))
        un2 = e(nc.sbuf_tensor("un2", [128, KC, 1], f32))
        xu2 = e(nc.sbuf_tensor("xu2", [128, KC, BS], mdt))
        wsq = e(nc.sbuf_tensor("wsq", [128, KC, C], mdt))
        R2 = e(nc.sbuf_tensor("R2", [128, KC, C], mdt))
        S_col = e(nc.sbuf_tensor("S_col", [128, 1], f32))
        scr_S = e(nc.sbuf_tensor("scr_S", [128, D], f32))
        wa_c = e(nc.sbuf_tensor("wa_c", [128, 2, C], f32))
        invS = e(nc.sbuf_tensor("invS", [128, 1], f32))
        t1s = e(nc.sbuf_tensor("t1s", [128, 1], f32))
        L = e(nc.sbuf_tensor("L", [128, C], f32))
        r = e(nc.sbuf_tensor("r", [128, C], f32))
        v = e(nc.sbuf_tensor("v", [128, C], f32))
        E1 = e(nc.sbuf_tensor("E1", [128, C], f32))
        s1 = e(nc.sbuf_tensor("s1", [128, 1], f32))
        wta = e(nc.sbuf_tensor("wta", [128, C], f32))
        r1 = e(nc.sbuf_tensor("r1", [128, 1], f32))
        E2 = e(nc.sbuf_tensor("E2", [128, C], f32))
        s2 = e(nc.sbuf_tensor("s2", [128, 1], f32))
        wf = e(nc.sbuf_tensor("wf", [128, 2, C], f32))
        r2 = e(nc.sbuf_tensor("r2", [128, 1], f32))
        yt = e(nc.sbuf_tensor("yt", [128, 2], f32))
        scr = e(nc.sbuf_tensor("scr", [128, 2, C], f32))
        rfin = e(nc.sbuf_tensor("rfin", [128, 1], f32))
        y_sb = e(nc.sbuf_tensor("y_sb", [128, 2], f32))

        # ---- PSUM (each a full 2KB bank) ----
        psum_ubc = e(nc.psum_tensor("psum_ubc", [128, C], f32))
        psum_wa0 = e(nc.psum_tensor("psum_wa0", [128, C], f32))
        psum_wa1 = e(nc.psum_tensor("psum_wa1", [128, C], f32))
        psum_mask = e(nc.psum_tensor("psum_mask", [128, C], f32))
        psum_main = e(nc.psum_tensor("psum_main", [128, C], f32))

        # views
        xT_sb = big_sb[:, :, 0:BS]
        wT_sb = big_sb[:, :, BS : BS + C]
        u_col = big_sb[:, :, BS + C : BS + C + 1]
        ones_blk = big_sb[:, 0, BS + C + 1 : W]          # [128,128] ones (f32r)
        mask_f = sm_sb[:, 0:C]
        wa_row = sm_sb[:, C : 3 * C]
        u_row = sm_sb[:, 3 * C : 3 * C + D]
        ones_row = sm_sb[:, 3 * C + D : SM]              # [1,128] ones (f32r)

        with nc.Block() as block:

            @block.sync
            def _(sync):
                nc.sync.dma_start(
                    out=big_sb[:, 0, :], in_=big.rearrange("(k p) n -> p k n", p=128)[:, 0, :]
                ).then_inc(s_big, 16)
                nc.sync.dma_start(
                    out=big_sb[:, 1, :], in_=big.rearrange("(k p) n -> p k n", p=128)[:, 1, :]
                ).then_inc(s_big, 16)
                sync.wait_ge(s_dve, 22)
                nc.sync.dma_start(out=y[:, :], in_=y_sb[:, :]).then_inc(s_out, 16)
                sync.wait_ge(s_out, 16)

            @block.scalar
            def _(scalar):
                a = 0
                nc.scalar.dma_start(out=sm_sb[:, :], in_=smalls[:, :]).then_inc(s_sm, 16)
                nc.scalar.dma_start(out=xn_sb[:, :], in_=xn[:, :]).then_inc(s_xn, 16)
                # 1: table warmup (single combined ln/exp set)
                scalar.wait_ge(s_dve, 1)
                nc.scalar.activation(warm[:, :], warm[:, :], AF.Ln).then_inc(s_act, 1)
                # 2,3: wsq = wT^2
                scalar.wait_ge(s_big, 32)
                for k in range(KC):
                    nc.scalar.activation(
                        wsq[:, k, :], wT_sb[:, k, :], AF.Square
                    ).then_inc(s_act, 1)
                # 4: S = sum u  (accumulate the PE broadcast of u)
                scalar.wait_ge(s_pe, 1)
                nc.scalar.activation(
                    scr_S[:, :], psum_ubc[:, 0:D], AF.Identity, accum_out=S_col[:, :]
                ).then_inc(s_act, 1)
                # 5: L = ln(psum*invS + t1s)
                scalar.wait_ge(s_pe, 9)
                scalar.wait_ge(s_dve, 13)
                nc.scalar.activation(
                    L[:, :], psum_main[:, :], AF.Ln, scale=invS[:, :], bias=t1s[:, :]
                ).then_inc(s_act, 1)
                # 6,7,8: r = exp(0.5 L); v = exp(-4r); E1 = exp(-3r) -> s1
                scalar.wait_ge(s_act, 5)
                nc.scalar.activation(r[:, :], L[:, :], AF.Exp, scale=0.5).then_inc(s_act, 1)
                scalar.wait_ge(s_act, 6)
                nc.scalar.activation(v[:, :], r[:, :], AF.Exp, scale=-4.0).then_inc(s_act, 1)
                nc.scalar.activation(
                    E1[:, :], r[:, :], AF.Exp, scale=-3.0, accum_out=s1[:, :]
                ).then_inc(s_act, 1)
                # 9: E2 = exp(wta/s1) -> s2
                scalar.wait_ge(s_dve, 15)
                nc.scalar.activation(
                    E2[:, :], wta[:, :], AF.Exp, scale=r1[:, :], accum_out=s2[:, :]
                ).then_inc(s_act, 1)

            @block.vector
            def _(vector):
                # 1: warm source
                nc.vector.memset(warm[:, :], 1.0).then_inc(s_dve, 1)
                # 2: mrow = BIG*(1-m)
                vector.wait_ge(s_sm, 16)
                nc.vector.tensor_scalar(
                    out=mrow[:, :], in0=mask_f, scalar1=-BIG, scalar2=BIG,
                    op0=OP.mult, op1=OP.add,
                ).then_inc(s_dve, 1)
                # 3: xx = x*x
                vector.wait_ge(s_xn, 16)
                nc.vector.tensor_mul(xx[:, :], xn_sb[:, :], xn_sb[:, :]).then_inc(s_dve, 1)
                # 4: t1 = sum_d u x^2 (fused accum vs the PE u-broadcast)
                vector.wait_ge(s_dve, 3)
                vector.wait_ge(s_pe, 1)
                nc.vector.scalar_tensor_tensor(
                    out=scr_t1[:, :], in0=xx[:, :], scalar=1.0, in1=psum_ubc[:, 0:D],
                    op0=OP.mult, op1=OP.mult, accum_out=t1c[:, :],
                ).then_inc(s_dve, 1)
                # 5: un2 = -2u ; 6,7: xu2
                vector.wait_ge(s_big, 32)
                nc.vector.tensor_scalar_mul(un2[:, :, :], dtv(u_col, f32), -2.0).then_inc(s_dve, 1)
                vector.wait_ge(s_dve, 5)
                for k in range(KC):
                    nc.vector.tensor_scalar_mul(
                        xu2[:, k, :], xT_sb[:, k, :], un2[:, k, :]
                    ).then_inc(s_dve, 1)
                # 8,9: R2 = u * wsq
                vector.wait_ge(s_act, 3)
                for k in range(KC):
                    nc.vector.tensor_scalar_mul(
                        R2[:, k, :], wsq[:, k, :], dtv(u_col[:, k, :], f32)
                    ).then_inc(s_dve, 1)
                # 10,11: stage w_assoc into SBUF
                vector.wait_ge(s_pe, 3)
                nc.vector.tensor_copy(wa_c[:, 0, :], psum_wa0[:, :]).then_inc(s_dve, 1)
                nc.vector.tensor_copy(wa_c[:, 1, :], psum_wa1[:, :]).then_inc(s_dve, 1)
                # 12,13: invS, t1s = (t1+eps)/S
                vector.wait_ge(s_act, 4)
                nc.vector.reciprocal(invS[:, :], S_col[:, :]).then_inc(s_dve, 1)
                vector.wait_ge(s_dve, 12)
                nc.vector.tensor_scalar(
                    out=t1s[:, :], in0=t1c[:, :], scalar1=EPS_RAW, scalar2=invS[:, :],
                    op0=OP.add, op1=OP.mult,
                ).then_inc(s_dve, 1)
                # 14: wta = 10v - BIGmask
                vector.wait_ge(s_act, 7)
                nc.vector.scalar_tensor_tensor(
                    out=wta[:, :], in0=v[:, :], scalar=10.0, in1=psum_mask[:, :],
                    op0=OP.mult, op1=OP.subtract,
                ).then_inc(s_dve, 1)
                # 15: r1 = 1/s1
                vector.wait_ge(s_act, 8)
                nc.vector.reciprocal(r1[:, :], s1[:, :]).then_inc(s_dve, 1)
                # 16,17: wf_j = v * wa_j
                vector.wait_ge(s_dve, 15)
                nc.vector.tensor_mul(wf[:, 0, :], v[:, :], wa_c[:, 0, :]).then_inc(s_dve, 1)
                nc.vector.tensor_mul(wf[:, 1, :], v[:, :], wa_c[:, 1, :]).then_inc(s_dve, 1)
                # 18: r2 ; 19,20: yt_j = sum_c 1.5*E2*wf_j
                vector.wait_ge(s_act, 9)
                nc.vector.reciprocal(r2[:, :], s2[:, :]).then_inc(s_dve, 1)
                vector.wait_ge(s_dve, 17)
                for j in range(2):
                    nc.vector.scalar_tensor_tensor(
                        out=scr[:, j, :], in0=E2[:, :], scalar=1.5, in1=wf[:, j, :],
                        op0=OP.mult, op1=OP.mult, accum_out=yt[:, j : j + 1],
                    ).then_inc(s_dve, 1)
                # 21,22: y = yt/(s1*s2)
                vector.wait_ge(s_dve, 18)
                nc.vector.tensor_scalar_mul(rfin[:, :], r1[:, :], r2[:, :]).then_inc(s_dve, 1)
                vector.wait_ge(s_dve, 21)
                nc.vector.tensor_scalar_mul(y_sb[:, :], yt[:, :], rfin[:, :]).then_inc(s_dve, 1)

            @block.tensor
            def _(tensor):
                # 1: u broadcast ; 2,3: w_assoc bcast ; 4: mask bcast
                tensor.wait_ge(s_sm, 16)
                nc.tensor.matmul(
                    psum_ubc[:, 0:D], lhsT=ones_row, rhs=u_row, start=True, stop=True
                ).then_inc(s_pe, 1)
                nc.tensor.matmul(
                    psum_wa0[:, :], lhsT=ones_row, rhs=wa_row[:, 0:C], start=True, stop=True
                ).then_inc(s_pe, 1)
                nc.tensor.matmul(
                    psum_wa1[:, :], lhsT=ones_row, rhs=wa_row[:, C : 2 * C], start=True, stop=True
                ).then_inc(s_pe, 1)
                tensor.wait_ge(s_dve, 2)
                nc.tensor.matmul(
                    psum_mask[:, :], lhsT=ones_row, rhs=mrow[:, :], start=True, stop=True
                ).then_inc(s_pe, 1)
                # 5..9: main accumulation
                tensor.wait_ge(s_dve, 7)
                for k in range(KC):
                    nc.tensor.matmul(
                        psum_main[:, :], lhsT=xu2[:, k, :], rhs=wT_sb[:, k, :],
                        start=(k == 0), stop=False,
                    ).then_inc(s_pe, 1)
                tensor.wait_ge(s_dve, 9)
                for k in range(KC):
                    nc.tensor.matmul(
                        psum_main[:, :], lhsT=ones_blk, rhs=R2[:, k, :],
                        start=False, stop=False,
                    ).then_inc(s_pe, 1)
                nc.tensor.matmul(
                    psum_main[:, :], lhsT=ones_row, rhs=mrow[:, :], start=False, stop=True
                ).then_inc(s_pe, 1)

            if end_clear:

                @block.gpsimd
                def _(gpsimd):
                    gpsimd.wait_ge(s_out, 16)
                    lo = min(s.num for s in (s_big, s_sm, s_xn, s_dve, s_act, s_pe, s_out))
                    hi = max(s.num for s in (s_big, s_sm, s_xn, s_dve, s_act, s_pe, s_out))
                    nc.gpsimd.dma_reset(range(lo, hi + 1))
                    nc.gpsimd.sem_clear(range(lo, hi + 1))

    nc.compile()
    return nc


def _get_nc(matmul_dt_name="float32r"):
    if matmul_dt_name not in _CACHE:
        _CACHE[matmul_dt_name] = _build(matmul_dt_name)
    return _CACHE[matmul_dt_name]


def kernel(inp, w_dist, attn, w_assoc, mask, _trace=False, _tmpdir=None,
           _matmul_dt="float32r"):
    from concourse.bass_utils import run_bass_kernel_spmd

    inp = np.asarray(inp, dtype=np.float32)
    w_dist = np.asarray(w_dist, dtype=np.float32)
    attn = np.asarray(attn, dtype=np.float32)
    w_assoc = np.asarray(w_assoc, dtype=np.float32)
    mask = np.asarray(mask, dtype=np.int32)

    # host-side layout prep only: transpose / concat / shard (+ ones blocks)
    xT_full = inp.T                                   # [D, B]
    wT = w_dist.T                                     # [D, C]
    u_col = attn.reshape(D, 1)
    ones_cols = np.ones((D, 128), dtype=np.float32)
    smalls = np.concatenate(
        [
            mask.astype(np.float32),
            w_assoc.T.reshape(-1).astype(np.float32),
            attn,
            np.ones(128, dtype=np.float32),
        ]
    ).reshape(1, SM)
    smalls = np.ascontiguousarray(smalls, dtype=np.float32)

    nc = _get_nc(_matmul_dt)

    in_maps = []
    for i in range(N_CORES):
        bigi = np.ascontiguousarray(
            np.concatenate(
                [xT_full[:, i * BS : (i + 1) * BS], wT, u_col, ones_cols], axis=1
            )
        )
        xni = np.ascontiguousarray(inp[i * BS : (i + 1) * BS, :])
        in_maps.append({"big": bigi, "xn": xni, "smalls": smalls})

    kw = {}
    if _trace:
        kw["trace"] = True
        if _tmpdir:
            kw["tmpdir"] = _tmpdir
    res = run_bass_kernel_spmd(nc, in_maps, core_ids=list(range(N_CORES)), **kw)
    out = np.concatenate([res.results[i]["y"] for i in range(N_CORES)], axis=0)
    if _trace:
        return out.astype(np.float32), res
    return out.astype(np.float32)


# revision 21
# speedup vs baseline: 1.0233x; 1.0233x over previous
"""Trainium2 Bass kernel for nn_ClusteringModel (vq_codebook).

Reference math (R=2, Q=1, c=1, beta=3, Tc=1, Twta=0.1, phi=1.5):
  a = attn/S;  wdist_bc = sum_d a_d (x_bd - w_cd)^2;  r = sqrt(wdist)
  p_comp = softmax_c(-3r | recruited); competed = p_comp * exp(-r) * m
  p_wta  = softmax_c(competed/0.1 | recruited)
  y = 1.5 * (p_wta * competed) @ w_assoc

Kernel algebra (u = raw attn, S = sum u):
  wdist*S = sum_d u x^2 - 2 sum_d u x w + sum_d u w^2  as ONE PSUM
  accumulation: the cross term is a K=256 float32r matmul, u*w^2 rides a
  ones-block lhsT, and the recruitment mask enters additively (+BIG) via a
  K=1 ones-row matmul (which doubles as a partition broadcast).
  r = exp(0.5*ln(wdist)) keeps every ACT func (ln/exp/square) in one
  activation-table set -> a single table load.  E1 = exp(-3r) -> s1;
  v = exp(-4r);  E2 = exp((10v - BIGmask)/s1) -> s2;
  y = 1.5/(s1*s2) * (E2*v) @ w_assoc.

This version is RAW bacc (no TileContext): 4 hand-scheduled engine streams
(Sync/Scalar handle DMA issue, PE, DVE, ACT) with 7 monotonic semaphores
and at most one wait per instruction (extra deps use standalone waits), so
the TRN2 event-semaphore splitting pass has nothing to do and the kernel
carries no scheduler / sem-cleanup overhead.

Sharding: data-parallel over batch (8 cores x 128 rows); w_dist/attn/
w_assoc/mask replicated. Host does layout prep only (transpose/concat).
"""

import sys

if "/opt/trn_rl_repo" not in sys.path:
    sys.path.insert(0, "/opt/trn_rl_repo")

import numpy as np

B, C, D = 1024, 512, 256
N_CORES = 8
BS = B // N_CORES            # 128 batch rows per core
KC = D // 128                # 2 contraction chunks
W = BS + C + 1               # big row: [xT | wT | u]
SM = 3 * C + D + 128         # smalls:  [mask | waT | u | ones_row]
BIG = 1.0e8
EPS_RAW = 0.01

_CACHE = {}
_PATCHED = False


def _apply_env_patches():
    """Collapse the ln/exp activation-table choice onto the combined set so
    the kernel needs exactly one ACT table load."""
    global _PATCHED
    if _PATCHED:
        return
    import copy

    import concourse.bacc as bacc
    import concourse.mybir as mybir

    AF = mybir.ActivationFunctionType
    orig_tables = bacc.get_activation_tables

    def tables_single_ln_exp(module_arch):
        t = copy.deepcopy(orig_tables(module_arch))
        for name, funcs in t.items():
            if name == "natural_log_exp_and_others":
                continue
            funcs.clear()
        return t

    bacc.get_activation_tables = tables_single_ln_exp
    _PATCHED = True


def _build(matmul_dt_name="float32r", end_clear=False):
    import dataclasses
    from contextlib import ExitStack

    import concourse.bacc as bacc
    import concourse.mybir as mybir

    _apply_env_patches()

    mdt = getattr(mybir.dt, matmul_dt_name)
    f32 = mybir.dt.float32
    AF = mybir.ActivationFunctionType
    OP = mybir.AluOpType

    def dtv(ap, dt):
        if ap.tensor.dtype == dt:
            return ap
        return dataclasses.replace(ap, tensor=dataclasses.replace(ap.tensor, dtype=dt))

    nc = bacc.Bacc("TRN2", target_bir_lowering=False)

    big = nc.dram_tensor("big", [D, W], mdt, kind="ExternalInput")
    xn = nc.dram_tensor("xn", [BS, D], f32, kind="ExternalInput")
    smalls = nc.dram_tensor("smalls", [1, SM], mdt, kind="ExternalInput")
    y = nc.dram_tensor("y", [BS, 2], f32, kind="ExternalOutput")

    with ExitStack() as ctx:
        e = ctx.enter_context

        # ---- semaphores (monotonic; runtime provides them zeroed) ----
        s_big = e(nc.semaphore("s_big"))
        s_sm = e(nc.semaphore("s_sm"))
        s_xn = e(nc.semaphore("s_xn"))
        s_dve = e(nc.semaphore("s_dve"))
        s_act = e(nc.semaphore("s_act"))
        s_pe = e(nc.semaphore("s_pe"))
        s_out = e(nc.semaphore("s_out"))

        # ---- SBUF ----
        big_sb = e(nc.sbuf_tensor("big_sb", [128, KC, W], mdt))
        sm_sb = e(nc.sbuf_tensor("sm_sb", [1, SM], mdt))
        xn_sb = e(nc.sbuf_tensor("xn_sb", [BS, D], f32))
        warm = e(nc.sbuf_tensor("warm", [1, 1], f32))
        mrow = e(nc.sbuf_tensor("mrow", [1, C], mdt))
        xx = e(nc.sbuf_tensor("xx", [BS, D], f32))
        t1c = e(nc.sbuf_tensor("t1c", [BS, 1], f32))
        scr_t1 = e(nc.sbuf_tensor("scr_t1", [BS, D], f32))
        un2 = e(nc.sbuf_tensor("un2", [128, KC, 1], f32))
        xu2 = e(nc.sbuf_tensor("xu2", [128, KC, BS], mdt))
        R2 = e(nc.sbuf_tensor("R2", [128, KC, C], mdt))
        S_col = e(nc.sbuf_tensor("S_col", [128, 1], f32))
        scr_S = e(nc.sbuf_tensor("scr_S", [128, D], f32))
        wa_c = e(nc.sbuf_tensor("wa_c", [128, 2, C], f32))
        invS = e(nc.sbuf_tensor("invS", [128, 1], f32))
        t1s = e(nc.sbuf_tensor("t1s", [128, 1], f32))
        L = e(nc.sbuf_tensor("L", [128, C], f32))
        r = e(nc.sbuf_tensor("r", [128, C], f32))
        v = e(nc.sbuf_tensor("v", [128, C], f32))
        E1 = e(nc.sbuf_tensor("E1", [128, C], f32))
        s1 = e(nc.sbuf_tensor("s1", [128, 1], f32))
        wta = e(nc.sbuf_tensor("wta", [128, C], f32))
        r1 = e(nc.sbuf_tensor("r1", [128, 1], f32))
        E2 = e(nc.sbuf_tensor("E2", [128, C], f32))
        s2 = e(nc.sbuf_tensor("s2", [128, 1], f32))
        wf = e(nc.sbuf_tensor("wf", [128, 2, C], f32))
        r2 = e(nc.sbuf_tensor("r2", [128, 1], f32))
        yt = e(nc.sbuf_tensor("yt", [128, 2], f32))
        scr = e(nc.sbuf_tensor("scr", [128, 2, C], f32))
        rfin = e(nc.sbuf_tensor("rfin", [128, 1], f32))
        y_sb = e(nc.sbuf_tensor("y_sb", [128, 2], f32))

        # ---- PSUM (each a full 2KB bank) ----
        psum_ubc = e(nc.psum_tensor("psum_ubc", [128, C], f32))
        psum_wa0 = e(nc.psum_tensor("psum_wa0", [128, C], f32))
        psum_wa1 = e(nc.psum_tensor("psum_wa1", [128, C], f32))
        psum_mask = e(nc.psum_tensor("psum_mask", [128, C], f32))
        psum_main = e(nc.psum_tensor("psum_main", [128, C], f32))

        # views
        xT_sb = big_sb[:, :, 0:BS]
        wT_sb = big_sb[:, :, BS : BS + C]
        u_col = big_sb[:, :, BS + C : BS + C + 1]
        ones_f32 = e(nc.sbuf_tensor("ones_f32", [128, 128], f32))
        ones_blk_t = e(nc.sbuf_tensor("ones_blk", [128, 128], mdt))
        ones_blk = ones_blk_t[:, :]
        su = e(nc.sbuf_tensor("su", [128, KC, 1], f32))
        sul = e(nc.sbuf_tensor("sul", [128, KC, 1], f32))
        mask_f = sm_sb[:, 0:C]
        wa_row = sm_sb[:, C : 3 * C]
        u_row = sm_sb[:, 3 * C : 3 * C + D]
        ones_row = sm_sb[:, 3 * C + D : SM]              # [1,128] ones (f32r)

        with nc.Block(no_gpsimd_drain=True) as block:

            @block.sync
            def _(sync):
                nc.sync.dma_start(
                    out=big_sb[:, 0, :], in_=big.rearrange("(k p) n -> p k n", p=128)[:, 0, :]
                ).then_inc(s_big, 16)
                nc.sync.dma_start(out=xn_sb[:, :], in_=xn[:, :]).then_inc(s_xn, 16)
                sync.wait_ge(s_dve, 21)
                nc.sync.dma_start(out=y[:, :], in_=y_sb[:, :]).then_inc(s_out, 16)
                sync.wait_ge(s_out, 16)

            @block.scalar
            def _(scalar):
                a = 0
                nc.scalar.dma_start(out=sm_sb[:, :], in_=smalls[:, :]).then_inc(s_sm, 16)
                nc.scalar.dma_start(
                    out=big_sb[:, 1, :], in_=big.rearrange("(k p) n -> p k n", p=128)[:, 1, :]
                ).then_inc(s_big, 16)
                # 1: table warmup (single combined set); 2: ones -> f32r
                scalar.wait_ge(s_dve, 1)
                nc.scalar.activation(warm[:, :], warm[:, :], AF.Ln).then_inc(s_act, 1)
                scalar.wait_ge(s_dve, 2)
                nc.scalar.copy(ones_blk, ones_f32[:, :]).then_inc(s_act, 1)
                # 3,4: su = sqrt(u) = exp(0.5 ln u)   [tiny, keeps one table set]
                scalar.wait_ge(s_big, 32)
                nc.scalar.activation(sul[:, :, :], dtv(u_col, f32), AF.Ln).then_inc(s_act, 1)
                scalar.wait_ge(s_act, 3)
                nc.scalar.activation(su[:, :, :], sul[:, :, :], AF.Exp, scale=0.5).then_inc(s_act, 1)
                # 5,6: R2 = (sqrt(u)*wT)^2 = u*wT^2
                scalar.wait_ge(s_act, 4)
                for k in range(KC):
                    nc.scalar.activation(
                        R2[:, k, :], wT_sb[:, k, :], AF.Square, scale=su[:, k, :]
                    ).then_inc(s_act, 1)
                # 7: S = sum u  (accumulate the PE broadcast of u)
                scalar.wait_ge(s_pe, 1)
                nc.scalar.activation(
                    scr_S[:, :], psum_ubc[:, 0:D], AF.Identity, accum_out=S_col[:, :]
                ).then_inc(s_act, 1)
                # 8: L = ln(psum*invS + t1s)
                scalar.wait_ge(s_pe, 9)
                scalar.wait_ge(s_dve, 12)
                nc.scalar.activation(
                    L[:, :], psum_main[:, :], AF.Ln, scale=invS[:, :], bias=t1s[:, :]
                ).then_inc(s_act, 1)
                # 9,10,11: r = exp(0.5 L); v = exp(-4r); E1 = exp(-3r) -> s1
                scalar.wait_ge(s_act, 8)
                nc.scalar.activation(r[:, :], L[:, :], AF.Exp, scale=0.5).then_inc(s_act, 1)
                scalar.wait_ge(s_act, 9)
                nc.scalar.activation(v[:, :], r[:, :], AF.Exp, scale=-4.0).then_inc(s_act, 1)
                nc.scalar.activation(
                    E1[:, :], r[:, :], AF.Exp, scale=-3.0, accum_out=s1[:, :]
                ).then_inc(s_act, 1)
                # 12: E2 = exp(wta/s1) -> s2
                scalar.wait_ge(s_dve, 14)
                nc.scalar.activation(
                    E2[:, :], wta[:, :], AF.Exp, scale=r1[:, :], accum_out=s2[:, :]
                ).then_inc(s_act, 1)

            @block.vector
            def _(vector):
                # 1: warm source ; 2: ones block (f32 side)
                nc.vector.memset(warm[:, :], 1.0).then_inc(s_dve, 1)
                nc.vector.memset(ones_f32[:, :], 1.0).then_inc(s_dve, 1)
                # 3: mrow = BIG*(1-m)
                vector.wait_ge(s_sm, 16)
                nc.vector.tensor_scalar(
                    out=mrow[:, :], in0=mask_f, scalar1=-BIG, scalar2=BIG,
                    op0=OP.mult, op1=OP.add,
                ).then_inc(s_dve, 1)
                # 4: xx = x*x
                vector.wait_ge(s_xn, 16)
                nc.vector.tensor_mul(xx[:, :], xn_sb[:, :], xn_sb[:, :]).then_inc(s_dve, 1)
                # 5: t1 = sum_d u x^2 (fused accum vs the PE u-broadcast)
                vector.wait_ge(s_dve, 4)
                vector.wait_ge(s_pe, 1)
                nc.vector.scalar_tensor_tensor(
                    out=scr_t1[:, :], in0=xx[:, :], scalar=1.0, in1=psum_ubc[:, 0:D],
                    op0=OP.mult, op1=OP.mult, accum_out=t1c[:, :],
                ).then_inc(s_dve, 1)
                # 6: un2 = -2u ; 7,8: xu2
                vector.wait_ge(s_big, 32)
                nc.vector.tensor_scalar_mul(un2[:, :, :], dtv(u_col, f32), -2.0).then_inc(s_dve, 1)
                vector.wait_ge(s_dve, 6)
                for k in range(KC):
                    nc.vector.tensor_scalar_mul(
                        xu2[:, k, :], xT_sb[:, k, :], un2[:, k, :]
                    ).then_inc(s_dve, 1)
                # 9,10: stage w_assoc into SBUF
                vector.wait_ge(s_pe, 3)
                nc.vector.tensor_copy(wa_c[:, 0, :], psum_wa0[:, :]).then_inc(s_dve, 1)
                nc.vector.tensor_copy(wa_c[:, 1, :], psum_wa1[:, :]).then_inc(s_dve, 1)
                # 11,12: invS, t1s = (t1+eps)/S
                vector.wait_ge(s_act, 7)
                nc.vector.reciprocal(invS[:, :], S_col[:, :]).then_inc(s_dve, 1)
                vector.wait_ge(s_dve, 11)
                nc.vector.tensor_scalar(
                    out=t1s[:, :], in0=t1c[:, :], scalar1=EPS_RAW, scalar2=invS[:, :],
                    op0=OP.add, op1=OP.mult,
                ).then_inc(s_dve, 1)
                # 13: wta = 10v - BIGmask
                vector.wait_ge(s_act, 10)
                nc.vector.scalar_tensor_tensor(
                    out=wta[:, :], in0=v[:, :], scalar=10.0, in1=psum_mask[:, :],
                    op0=OP.mult, op1=OP.subtract,
                ).then_inc(s_dve, 1)
                # 14: r1 = 1/s1
                vector.wait_ge(s_act, 11)
                nc.vector.reciprocal(r1[:, :], s1[:, :]).then_inc(s_dve, 1)
                # 15,16: wf_j = v * wa_j
                vector.wait_ge(s_dve, 14)
                nc.vector.tensor_mul(wf[:, 0, :], v[:, :], wa_c[:, 0, :]).then_inc(s_dve, 1)
                nc.vector.tensor_mul(wf[:, 1, :], v[:, :], wa_c[:, 1, :]).then_inc(s_dve, 1)
                # 17: r2 ; 18,19: yt_j
                vector.wait_ge(s_act, 12)
                nc.vector.reciprocal(r2[:, :], s2[:, :]).then_inc(s_dve, 1)
                vector.wait_ge(s_dve, 16)
                for j in range(2):
                    nc.vector.scalar_tensor_tensor(
                        out=scr[:, j, :], in0=E2[:, :], scalar=1.5, in1=wf[:, j, :],
                        op0=OP.mult, op1=OP.mult, accum_out=yt[:, j : j + 1],
                    ).then_inc(s_dve, 1)
                # 20,21: y = yt/(s1*s2)
                vector.wait_ge(s_dve, 17)
                nc.vector.tensor_scalar_mul(rfin[:, :], r1[:, :], r2[:, :]).then_inc(s_dve, 1)
                vector.wait_ge(s_dve, 20)
                nc.vector.tensor_scalar_mul(y_sb[:, :], yt[:, :], rfin[:, :]).then_inc(s_dve, 1)

            @block.tensor
            def _(tensor):
                # 1: u broadcast ; 2,3: w_assoc bcast ; 4: mask bcast
                tensor.wait_ge(s_sm, 16)
                nc.tensor.matmul(
                    psum_ubc[:, 0:D], lhsT=ones_row, rhs=u_row, start=True, stop=True
                ).then_inc(s_pe, 1)
                nc.tensor.matmul(
                    psum_wa0[:, :], lhsT=ones_row, rhs=wa_row[:, 0:C], start=True, stop=True
                ).then_inc(s_pe, 1)
                nc.tensor.matmul(
                    psum_wa1[:, :], lhsT=ones_row, rhs=wa_row[:, C : 2 * C], start=True, stop=True
                ).then_inc(s_pe, 1)
                tensor.wait_ge(s_dve, 3)
                nc.tensor.matmul(
                    psum_mask[:, :], lhsT=ones_row, rhs=mrow[:, :], start=True, stop=True
                ).then_inc(s_pe, 1)
                # 5..9: main accumulation
                tensor.wait_ge(s_dve, 8)
                for k in range(KC):
                    nc.tensor.matmul(
                        psum_main[:, :], lhsT=xu2[:, k, :], rhs=wT_sb[:, k, :],
                        start=(k == 0), stop=False,
                    ).then_inc(s_pe, 1)
                tensor.wait_ge(s_act, 6)
                for k in range(KC):
                    nc.tensor.matmul(
                        psum_main[:, :], lhsT=ones_blk, rhs=R2[:, k, :],
                        start=False, stop=False,
                    ).then_inc(s_pe, 1)
                nc.tensor.matmul(
                    psum_main[:, :], lhsT=ones_row, rhs=mrow[:, :], start=False, stop=True
                ).then_inc(s_pe, 1)

            if end_clear:

                @block.gpsimd
                def _(gpsimd):
                    gpsimd.wait_ge(s_out, 16)
                    lo = min(s.num for s in (s_big, s_sm, s_xn, s_dve, s_act, s_pe, s_out))
                    hi = max(s.num for s in (s_big, s_sm, s_xn, s_dve, s_act, s_pe, s_out))
                    nc.gpsimd.dma_reset(range(lo, hi + 1))
                    nc.gpsimd.sem_clear(range(lo, hi + 1))

    nc.compile()
    return nc


def _get_nc(matmul_dt_name="float32r"):
    if matmul_dt_name not in _CACHE:
        _CACHE[matmul_dt_name] = _build(matmul_dt_name)
    return _CACHE[matmul_dt_name]


def kernel(inp, w_dist, attn, w_assoc, mask, _trace=False, _tmpdir=None,
           _matmul_dt="float32r"):
    from concourse.bass_utils import run_bass_kernel_spmd

    inp = np.asarray(inp, dtype=np.float32)
    w_dist = np.asarray(w_dist, dtype=np.float32)
    attn = np.asarray(attn, dtype=np.float32)
    w_assoc = np.asarray(w_assoc, dtype=np.float32)
    mask = np.asarray(mask, dtype=np.int32)

    # host-side layout prep only: transpose / concat / shard (+ ones blocks)
    xT_full = inp.T                                   # [D, B]
    wT = w_dist.T                                     # [D, C]
    u_col = attn.reshape(D, 1)
    smalls = np.concatenate(
        [
            mask.astype(np.float32),
            w_assoc.T.reshape(-1).astype(np.float32),
            attn,
            np.ones(128, dtype=np.float32),
        ]
    ).reshape(1, SM)
    smalls = np.ascontiguousarray(smalls, dtype=np.float32)

    nc = _get_nc(_matmul_dt)

    in_maps = []
    for i in range(N_CORES):
        bigi = np.ascontiguousarray(
            np.concatenate(
                [xT_full[:, i * BS : (i + 1) * BS], wT, u_col], axis=1
            )
        )
        xni = np.ascontiguousarray(inp[i * BS : (i + 1) * BS, :])
        in_maps.append({"big": bigi, "xn": xni, "smalls": smalls})

    kw = {}
    if _trace:
        kw["trace"] = True
        if _tmpdir:
            kw["tmpdir"] = _tmpdir
    res = run_bass_kernel_spmd(nc, in_maps, core_ids=list(range(N_CORES)), **kw)
    out = np.concatenate([res.results[i]["y"] for i in range(N_CORES)], axis=0)
    if _trace:
        return out.astype(np.float32), res
    return out.astype(np.float32)


# revision 22
# speedup vs baseline: 1.0515x; 1.0275x over previous
"""Trainium2 Bass kernel for nn_ClusteringModel (vq_codebook).

Reference math (R=2, Q=1, c=1, beta=3, Tc=1, Twta=0.1, phi=1.5):
  a = attn/S;  wdist_bc = sum_d a_d (x_bd - w_cd)^2;  r = sqrt(wdist)
  p_comp = softmax_c(-3r | recruited); competed = p_comp * exp(-r) * m
  p_wta  = softmax_c(competed/0.1 | recruited)
  y = 1.5 * (p_wta * competed) @ w_assoc

Kernel algebra (u = raw attn, S = sum u):
  wdist*S = sum_d u x^2 - 2 sum_d u x w + sum_d u w^2  as ONE PSUM
  accumulation: cross term = K=256 float32r matmul; u*w^2 rides a
  ones-block lhsT; the mask enters additively (+BIG) via a K=1 ones-row
  matmul (doubling as a partition broadcast).  r = exp(0.5*ln(wdist)) and
  sqrt(u) = exp(0.5*ln(u)) keep every ACT func (ln/exp/square/copy/id)
  inside ONE activation-table set -> a single early table load.
  E1 = exp(-3r) -> s1;  v = exp(-4r);  E2 = exp((10v - BIGmask)/s1) -> s2;
  y = 1.5/(s1*s2) * (E2*v) @ w_assoc.

RAW bacc implementation (no TileContext): hand-scheduled engine streams
(Sync+Scalar issue DMAs, PE, DVE, ACT) with 8 monotonic semaphores and at
most one wait per instruction (extra deps become standalone waits), so the
TRN2 event-sem splitting pass stays idle and there is no scheduler or
semaphore-cleanup overhead. All activations use an explicit zero-bias tile
so nothing reads the preamble const pool, which lets the init barrier be
sem-only (no DRAIN inside the profiled window).

Sharding: data-parallel over batch (8 cores x 128 rows); w_dist/attn/
w_assoc/mask replicated. Host does layout prep only (transpose/concat).
"""

import sys

if "/opt/trn_rl_repo" not in sys.path:
    sys.path.insert(0, "/opt/trn_rl_repo")

import numpy as np

B, C, D = 1024, 512, 256
N_CORES = 8
BS = B // N_CORES            # 128 batch rows per core
KC = D // 128                # 2 contraction chunks
W = BS + C + 1               # big row: [xT | wT | u]
SM = 3 * C + D + 128         # smalls:  [mask | waT | u | ones_row]
BIG = 1.0e8                  # masked wdist ~ BIG/S -> r ~ 900 -> exp -> 0;
                             # stays inside the Ln table domain (2^64)
EPS_RAW = 0.01               # keeps the ln argument strictly positive

# static per-engine instruction indices (value of the engine's semaphore
# after the op completes)
ACT = dict(warm=1, ones=2, sul0=3, su0=4, R20=5, sul1=6, su1=7, R21=8,
           S_col=9, L=10, r=11, v=12, E1=13, E2=14)
DVE = dict(warm=1, zeros=2, ones_f32=3, mrow=4, xx=5, t1c=6, un20=7, xu20=8,
           un21=9, xu21=10, wac0=11, wac1=12, invS=13, t1s=14, wta=15, r1=16,
           wf0=17, wf1=18, r2=19, stt0=20, stt1=21, rfin=22, y_sb=23)
PE = dict(ubc=1, wa0=2, wa1=3, mask=4, xw0=5, xw1=6, R2c0=7, R2c1=8, main=9)

_CACHE = {}
_PATCHED = False


def _apply_env_patches():
    """Make the act-table pass see only the combined ln/exp set so exactly
    one ACT table load is emitted (walrus still loads the real table)."""
    global _PATCHED
    if _PATCHED:
        return
    import copy

    import concourse.bacc as bacc

    orig_tables = bacc.get_activation_tables

    def tables_single_set(module_arch):
        t = copy.deepcopy(orig_tables(module_arch))
        for name, funcs in t.items():
            if name != "natural_log_exp_and_others":
                funcs.clear()
        return t

    bacc.get_activation_tables = tables_single_set
    _PATCHED = True


def _build(matmul_dt_name="float32r", out_swdge=False):
    import dataclasses
    from contextlib import ExitStack

    import concourse.bacc as bacc
    import concourse.mybir as mybir

    _apply_env_patches()

    mdt = getattr(mybir.dt, matmul_dt_name)
    f32 = mybir.dt.float32
    AF = mybir.ActivationFunctionType
    OP = mybir.AluOpType

    def dtv(ap, dt):
        if ap.tensor.dtype == dt:
            return ap
        return dataclasses.replace(ap, tensor=dataclasses.replace(ap.tensor, dtype=dt))

    # The init barrier only orders the preamble const memsets, which nothing
    # reads (all biases are explicit APs): sem-only keeps DRAIN out of the
    # profiled window.
    _orig_aeb = bacc.Bacc.all_engine_barrier
    bacc.Bacc.all_engine_barrier = lambda self, **kw: _orig_aeb(self, sem_only=True)
    try:
        nc = bacc.Bacc("TRN2", target_bir_lowering=False)
    finally:
        bacc.Bacc.all_engine_barrier = _orig_aeb

    big = nc.dram_tensor("big", [D, W], mdt, kind="ExternalInput")
    xn = nc.dram_tensor("xn", [BS, D], f32, kind="ExternalInput")
    smalls = nc.dram_tensor("smalls", [1, SM], mdt, kind="ExternalInput")
    y = nc.dram_tensor("y", [BS, 2], f32, kind="ExternalOutput")

    with ExitStack() as ctx:
        e = ctx.enter_context

        s_big0 = e(nc.semaphore("s_big0"))
        s_big1 = e(nc.semaphore("s_big1"))
        s_sm = e(nc.semaphore("s_sm"))
        s_xn = e(nc.semaphore("s_xn"))
        s_dve = e(nc.semaphore("s_dve"))
        s_act = e(nc.semaphore("s_act"))
        s_pe = e(nc.semaphore("s_pe"))
        s_out = e(nc.semaphore("s_out"))

        def sb(name, shape, dt=f32):
            return e(nc.sbuf_tensor(name, shape, dt))

        big_sb = sb("big_sb", [128, KC, W], mdt)
        sm_sb = sb("sm_sb", [1, SM], mdt)
        xn_sb = sb("xn_sb", [BS, D])
        warm = sb("warm", [1, 1])
        zeros = sb("zeros", [128, 1])
        ones_f32 = sb("ones_f32", [128, 128])
        ones_blk = sb("ones_blk", [128, 128], mdt)
        mrow = sb("mrow", [1, C], mdt)
        xx = sb("xx", [BS, D])
        t1c = sb("t1c", [BS, 1])
        scr_t1 = sb("scr_t1", [BS, D])
        un2 = sb("un2", [128, KC, 1])
        xu2 = sb("xu2", [128, KC, BS], mdt)
        sul = sb("sul", [128, KC, 1])
        su = sb("su", [128, KC, 1])
        R2 = sb("R2", [128, KC, C], mdt)
        S_col = sb("S_col", [128, 1])
        scr_S = sb("scr_S", [128, D])
        wa_c = sb("wa_c", [128, 2, C])
        invS = sb("invS", [128, 1])
        t1s = sb("t1s", [128, 1])
        L = sb("L", [128, C])
        r = sb("r", [128, C])
        v = sb("v", [128, C])
        E1 = sb("E1", [128, C])
        s1 = sb("s1", [128, 1])
        wta = sb("wta", [128, C])
        r1 = sb("r1", [128, 1])
        E2 = sb("E2", [128, C])
        s2 = sb("s2", [128, 1])
        wf = sb("wf", [128, 2, C])
        r2 = sb("r2", [128, 1])
        yt = sb("yt", [128, 2])
        scr = sb("scr", [128, 2, C])
        rfin = sb("rfin", [128, 1])
        y_sb = sb("y_sb", [128, 2])

        psum_ubc = e(nc.psum_tensor("psum_ubc", [128, C], f32))
        psum_wa0 = e(nc.psum_tensor("psum_wa0", [128, C], f32))
        psum_wa1 = e(nc.psum_tensor("psum_wa1", [128, C], f32))
        psum_mask = e(nc.psum_tensor("psum_mask", [128, C], f32))
        psum_main = e(nc.psum_tensor("psum_main", [128, C], f32))

        xT_sb = big_sb[:, :, 0:BS]
        wT_sb = big_sb[:, :, BS : BS + C]
        u_col = big_sb[:, :, BS + C : W]
        mask_f = sm_sb[:, 0:C]
        wa_row = sm_sb[:, C : 3 * C]
        u_row = sm_sb[:, 3 * C : 3 * C + D]
        ones_row = sm_sb[:, 3 * C + D : SM]

        z128 = zeros[:, :]
        z1 = zeros[0:1, :]

        with nc.Block(no_gpsimd_drain=True) as block:

            @block.sync
            def _(sync):
                big_r = big.rearrange("(k p) n -> p k n", p=128)
                nc.sync.dma_start(out=big_sb[:, 0, :], in_=big_r[:, 0, :]).then_inc(s_big0, 16)
                nc.sync.dma_start(out=xn_sb[:, :], in_=xn[:, :]).then_inc(s_xn, 16)
                if not out_swdge:
                    sync.wait_ge(s_dve, DVE["y_sb"])
                    nc.sync.dma_start(out=y[:, :], in_=y_sb[:, :]).then_inc(s_out, 16)
                sync.wait_ge(s_out, 16)

            @block.scalar
            def _(scalar):
                big_r = big.rearrange("(k p) n -> p k n", p=128)
                nc.scalar.dma_start(out=sm_sb[:, :], in_=smalls[:, :]).then_inc(s_sm, 16)
                nc.scalar.dma_start(out=big_sb[:, 1, :], in_=big_r[:, 1, :]).then_inc(s_big1, 16)
                # table warmup + f32r ones block
                scalar.wait_ge(s_dve, DVE["zeros"])
                nc.scalar.activation(warm[:, :], warm[:, :], AF.Ln, bias=z1).then_inc(s_act, 1)
                scalar.wait_ge(s_dve, DVE["ones_f32"])
                nc.scalar.copy(ones_blk[:, :], ones_f32[:, :]).then_inc(s_act, 1)
                # su_k = sqrt(u_k) = exp(0.5 ln u_k); R2_k = (su_k * w_k)^2
                for k in range(KC):
                    scalar.wait_ge(s_big0 if k == 0 else s_big1, 16)
                    nc.scalar.activation(
                        sul[:, k, :], dtv(u_col[:, k, :], f32), AF.Ln, bias=z128
                    ).then_inc(s_act, 1)
                    scalar.wait_ge(s_act, ACT[f"sul{k}"])
                    nc.scalar.activation(
                        su[:, k, :], sul[:, k, :], AF.Exp, scale=0.5, bias=z128
                    ).then_inc(s_act, 1)
                    scalar.wait_ge(s_act, ACT[f"su{k}"])
                    nc.scalar.activation(
                        R2[:, k, :], wT_sb[:, k, :], AF.Square, scale=su[:, k, :], bias=z128
                    ).then_inc(s_act, 1)
                # S = sum u (accumulated from the PE u-broadcast)
                scalar.wait_ge(s_pe, PE["ubc"])
                nc.scalar.activation(
                    scr_S[:, :], psum_ubc[:, 0:D], AF.Identity, accum_out=S_col[:, :],
                    bias=z128,
                ).then_inc(s_act, 1)
                # L = ln(psum*invS + t1s); r = exp(L/2); v = exp(-4r); E1 -> s1
                scalar.wait_ge(s_pe, PE["main"])
                scalar.wait_ge(s_dve, DVE["t1s"])
                nc.scalar.activation(
                    L[:, :], psum_main[:, :], AF.Ln, scale=invS[:, :], bias=t1s[:, :]
                ).then_inc(s_act, 1)
                scalar.wait_ge(s_act, ACT["L"])
                nc.scalar.activation(r[:, :], L[:, :], AF.Exp, scale=0.5, bias=z128).then_inc(s_act, 1)
                scalar.wait_ge(s_act, ACT["r"])
                nc.scalar.activation(v[:, :], r[:, :], AF.Exp, scale=-4.0, bias=z128).then_inc(s_act, 1)
                nc.scalar.activation(
                    E1[:, :], r[:, :], AF.Exp, scale=-3.0, bias=z128, accum_out=s1[:, :]
                ).then_inc(s_act, 1)
                scalar.wait_ge(s_dve, DVE["r1"])
                nc.scalar.activation(
                    E2[:, :], wta[:, :], AF.Exp, scale=r1[:, :], bias=z128,
                    accum_out=s2[:, :],
                ).then_inc(s_act, 1)

            @block.vector
            def _(vector):
                nc.vector.memset(warm[:, :], 1.0).then_inc(s_dve, 1)
                nc.vector.memset(zeros[:, :], 0.0).then_inc(s_dve, 1)
                nc.vector.memset(ones_f32[:, :], 1.0).then_inc(s_dve, 1)
                vector.wait_ge(s_sm, 16)
                nc.vector.tensor_scalar(
                    out=mrow[:, :], in0=mask_f, scalar1=-BIG, scalar2=BIG,
                    op0=OP.mult, op1=OP.add,
                ).then_inc(s_dve, 1)
                vector.wait_ge(s_xn, 16)
                nc.vector.tensor_mul(xx[:, :], xn_sb[:, :], xn_sb[:, :]).then_inc(s_dve, 1)
                vector.wait_ge(s_pe, PE["ubc"])
                vector.wait_ge(s_dve, DVE["xx"])
                nc.vector.scalar_tensor_tensor(
                    out=scr_t1[:, :], in0=xx[:, :], scalar=1.0, in1=psum_ubc[:, 0:D],
                    op0=OP.mult, op1=OP.mult, accum_out=t1c[:, :],
                ).then_inc(s_dve, 1)
                # per-chunk un2 / xu2 (chunk 0 starts before big chunk 1 lands)
                for k in range(KC):
                    vector.wait_ge(s_big0 if k == 0 else s_big1, 16)
                    nc.vector.tensor_scalar_mul(
                        un2[:, k, :], dtv(u_col[:, k, :], f32), -2.0
                    ).then_inc(s_dve, 1)
                    vector.wait_ge(s_dve, DVE[f"un2{k}"])
                    nc.vector.tensor_scalar_mul(
                        xu2[:, k, :], xT_sb[:, k, :], un2[:, k, :]
                    ).then_inc(s_dve, 1)
                vector.wait_ge(s_pe, PE["wa1"])
                nc.vector.tensor_copy(wa_c[:, 0, :], psum_wa0[:, :]).then_inc(s_dve, 1)
                nc.vector.tensor_copy(wa_c[:, 1, :], psum_wa1[:, :]).then_inc(s_dve, 1)
                vector.wait_ge(s_act, ACT["S_col"])
                nc.vector.reciprocal(invS[:, :], S_col[:, :]).then_inc(s_dve, 1)
                vector.wait_ge(s_dve, DVE["invS"])
                nc.vector.tensor_scalar(
                    out=t1s[:, :], in0=t1c[:, :], scalar1=EPS_RAW, scalar2=invS[:, :],
                    op0=OP.add, op1=OP.mult,
                ).then_inc(s_dve, 1)
                vector.wait_ge(s_act, ACT["v"])
                nc.vector.scalar_tensor_tensor(
                    out=wta[:, :], in0=v[:, :], scalar=10.0, in1=psum_mask[:, :],
                    op0=OP.mult, op1=OP.subtract,
                ).then_inc(s_dve, 1)
                vector.wait_ge(s_act, ACT["E1"])
                nc.vector.reciprocal(r1[:, :], s1[:, :]).then_inc(s_dve, 1)
                nc.vector.tensor_mul(wf[:, 0, :], v[:, :], wa_c[:, 0, :]).then_inc(s_dve, 1)
                nc.vector.tensor_mul(wf[:, 1, :], v[:, :], wa_c[:, 1, :]).then_inc(s_dve, 1)
                vector.wait_ge(s_act, ACT["E2"])
                nc.vector.reciprocal(r2[:, :], s2[:, :]).then_inc(s_dve, 1)
                vector.wait_ge(s_dve, DVE["wf1"])
                for j in range(2):
                    nc.vector.scalar_tensor_tensor(
                        out=scr[:, j, :], in0=E2[:, :], scalar=1.5, in1=wf[:, j, :],
                        op0=OP.mult, op1=OP.mult, accum_out=yt[:, j : j + 1],
                    ).then_inc(s_dve, 1)
                vector.wait_ge(s_dve, DVE["r2"])
                nc.vector.tensor_scalar_mul(rfin[:, :], r1[:, :], r2[:, :]).then_inc(s_dve, 1)
                vector.wait_ge(s_dve, DVE["rfin"])
                nc.vector.tensor_scalar_mul(y_sb[:, :], yt[:, :], rfin[:, :]).then_inc(s_dve, 1)

            @block.tensor
            def _(tensor):
                tensor.wait_ge(s_sm, 16)
                nc.tensor.matmul(
                    psum_ubc[:, 0:D], lhsT=ones_row, rhs=u_row, start=True, stop=True
                ).then_inc(s_pe, 1)
                nc.tensor.matmul(
                    psum_wa0[:, :], lhsT=ones_row, rhs=wa_row[:, 0:C], start=True, stop=True
                ).then_inc(s_pe, 1)
                nc.tensor.matmul(
                    psum_wa1[:, :], lhsT=ones_row, rhs=wa_row[:, C : 2 * C],
                    start=True, stop=True,
                ).then_inc(s_pe, 1)
                tensor.wait_ge(s_dve, DVE["mrow"])
                nc.tensor.matmul(
                    psum_mask[:, :], lhsT=ones_row, rhs=mrow[:, :], start=True, stop=True
                ).then_inc(s_pe, 1)
                for k in range(KC):
                    tensor.wait_ge(s_dve, DVE[f"xu2{k}"])
                    nc.tensor.matmul(
                        psum_main[:, :], lhsT=xu2[:, k, :], rhs=wT_sb[:, k, :],
                        start=(k == 0), stop=False,
                    ).then_inc(s_pe, 1)
                for k in range(KC):
                    tensor.wait_ge(s_act, ACT[f"R2{k}"])
                    nc.tensor.matmul(
                        psum_main[:, :], lhsT=ones_blk[:, :], rhs=R2[:, k, :],
                        start=False, stop=False,
                    ).then_inc(s_pe, 1)
                nc.tensor.matmul(
                    psum_main[:, :], lhsT=ones_row, rhs=mrow[:, :], start=False, stop=True
                ).then_inc(s_pe, 1)

            if out_swdge:

                @block.gpsimd
                def _(gpsimd):
                    gpsimd.wait_ge(s_dve, DVE["y_sb"])
                    nc.gpsimd.dma_start(out=y[:, :], in_=y_sb[:, :]).then_inc(s_out, 16)

    nc.compile()
    return nc


def _get_nc(matmul_dt_name="float32r", out_swdge=False):
    key = (matmul_dt_name, out_swdge)
    if key not in _CACHE:
        _CACHE[key] = _build(matmul_dt_name, out_swdge)
    return _CACHE[key]


def kernel(inp, w_dist, attn, w_assoc, mask, _trace=False, _tmpdir=None,
           _matmul_dt="float32r", _out_swdge=False):
    from concourse.bass_utils import run_bass_kernel_spmd

    inp = np.asarray(inp, dtype=np.float32)
    w_dist = np.asarray(w_dist, dtype=np.float32)
    attn = np.asarray(attn, dtype=np.float32)
    w_assoc = np.asarray(w_assoc, dtype=np.float32)
    mask = np.asarray(mask, dtype=np.int32)

    # host-side layout prep only: transpose / concat / shard
    xT_full = inp.T
    wT = w_dist.T
    u_col = attn.reshape(D, 1)
    smalls = np.concatenate(
        [
            mask.astype(np.float32),
            w_assoc.T.reshape(-1).astype(np.float32),
            attn,
            np.ones(128, dtype=np.float32),
        ]
    ).reshape(1, SM)
    smalls = np.ascontiguousarray(smalls, dtype=np.float32)

    nc = _get_nc(_matmul_dt, _out_swdge)

    in_maps = []
    for i in range(N_CORES):
        bigi = np.ascontiguousarray(
            np.concatenate([xT_full[:, i * BS : (i + 1) * BS], wT, u_col], axis=1)
        )
        xni = np.ascontiguousarray(inp[i * BS : (i + 1) * BS, :])
        in_maps.append({"big": bigi, "xn": xni, "smalls": smalls})

    kw = {}
    if _trace:
        kw["trace"] = True
        if _tmpdir:
            kw["tmpdir"] = _tmpdir
    res = run_bass_kernel_spmd(nc, in_maps, core_ids=list(range(N_CORES)), **kw)
    out = np.concatenate([res.results[i]["y"] for i in range(N_CORES)], axis=0)
    if _trace:
        return out.astype(np.float32), res
    return out.astype(np.float32)
